# revision 28
# baseline (speedup 1.0000x reference)
"""Trainium2 Bass kernel for nn_NeuralODE_Latent_MLP_10350871183740.

Data-parallel over batch: 2048 samples -> 8 cores x 256 samples.
Per core, two pipelined groups of 128 samples (features-on-partition,
batch-on-free).  Dopri5 stage combinations are folded into the L1 matmul
stationary (augmented-K trick); the step size h is folded into the
PSUM->SBUF copy of each k.  Inner loop runs in fp16 (PE 1 cyc/row) with
an fp32 y accumulator; validated numerically to sit at the fp32 noise
floor (~1.8e-3 rel-to-scale vs fp64).
"""
import sys

sys.path.insert(0, "/opt/trn_rl_repo")
import numpy as np

N_CORES = 8
B, T, OB, AC, OBL, ACL, W = 2048, 128, 64, 8, 32, 16, 64
BPC = B // N_CORES          # 256 samples per core
G = 2                       # pipelined groups per core
GB = BPC // G               # 128 samples per group
NI = T - 1                  # 127 intervals
GC = T * GB                 # 16384 columns per group (col = t*GB + s)
F16 = np.float16

# SBUF partition starts must be 0/32/64/96, so S1 is laid out as:
# y 0:32, A 32:48, ones 48, (unused 49:64), k1 64:96, k2 96:128.
SROWS = [49, 96, 128, 128, 128, 128]
B_ROWS = [0, 0, 0, 32, 64, 96]
B_COL = {3: 384, 4: 448, 5: 512}
DOPRI_A = [
    [],
    [1.0 / 5.0],
    [3.0 / 40.0, 9.0 / 40.0],
    [44.0 / 45.0, -56.0 / 15.0, 32.0 / 9.0],
    [19372.0 / 6561.0, -25360.0 / 2187.0, 64448.0 / 6561.0, -212.0 / 729.0],
    [9017.0 / 3168.0, -355.0 / 33.0, 46732.0 / 5247.0, 49.0 / 176.0,
     -5103.0 / 18656.0],
]
DOPRI_BW = [35.0 / 384.0, 0.0, 500.0 / 1113.0, 125.0 / 192.0,
            -2187.0 / 6784.0, 11.0 / 84.0]

_prog_cache: dict = {}
_JSON_PATCHED = False


_WAIT_LIMITS = {"Matmult": 1, "Ldweights": 1, "Activation": 1,
                "TensorCopy": 1, "TensorScalarPtr": 1, "Memset": 1,
                "TensorTensor": 1, "TensorReduce": 1, "DMACopy": 1}


def _split_mm_waits(bj: bytes) -> bytes:
    """Some ISA structs hold few sync-waits (Matmult: 1).  Hoist extras
    onto no-op Drain carriers on the same engine queue, inserted
    immediately before the instruction (in-order queue -> same
    semantics, no deadlock: nothing executes between carrier and
    target)."""
    import orjson

    m = orjson.loads(bj)
    changed = False
    for fn in m.get("functions", []):
        for blk in fn.get("blocks", []):
            out = []
            for ins in blk.get("instructions", []):
                si = ins.get("sync_info") or {}
                waits = si.get("on_wait") or []
                lim = _WAIT_LIMITS.get(ins.get("opcode"))
                if lim is not None and len(waits) > lim:
                    for k, w in enumerate(waits[lim:]):
                        out.append({
                            "debug": ins.get("debug", 0),
                            "engine": ins["engine"],
                            "ins": [],
                            "outs": [],
                            "name": f'{ins.get("name", "I")}-xw{k}',
                            "opcode": "Drain",
                            "sync_info": {"on_update": [], "on_wait": [w]},
                        })
                    si["on_wait"] = waits[:lim]
                    ins["sync_info"] = si
                    changed = True
                out.append(ins)
            blk["instructions"] = out
    return orjson.dumps(m) if changed else bj


def _patch_to_json(bass) -> None:
    global _JSON_PATCHED
    if _JSON_PATCHED:
        return
    _JSON_PATCHED = True
    orig = bass.Bass.to_json_bytes

    def patched(self):
        return _split_mm_waits(orig(self))

    bass.Bass.to_json_bytes = patched


def _schedule(t_row: np.ndarray):
    """Replicate the reference's fp32 stage-time / searchsorted math."""
    f32 = np.float32
    t_row = t_row.astype(f32)
    sched = []
    for i in range(NI):
        t0, t1 = t_row[i], t_row[i + 1]
        h = f32((t1 - t0) / f32(2.0))
        for j in range(2):
            tj = f32(t0 + f32(j) * h)
            ts = [
                tj,
                f32(tj + h * f32(1.0 / 5.0)),
                f32(tj + h * f32(3.0 / 10.0)),
                f32(tj + h * f32(4.0 / 5.0)),
                f32(tj + h * f32(8.0 / 9.0)),
                f32(tj + h),
            ]
            idxs = [
                int(np.clip(np.searchsorted(t_row, t, side="right") - 1, 0, T - 1))
                for t in ts
            ]
            sched.append((i, j, float(h), idxs))
    return sched


def _build_program(t_row: np.ndarray):
    import concourse.bass as bass
    import concourse.mybir as mybir
    from concourse.tile import TileContext
    from concourse.vector_clock import ScopedClock

    _patch_to_json(bass)

    class SplitDrainTileContext(TileContext):
        """Walrus rejects >N sem waits on one Drain; split them 1-per-drain."""

        def _drain_and_barrier(self, tick_clock, wait_clock):
            nc = self.nc
            drain_inst = nc.sync.drain()
            wait_clock.add_sem_waits(
                drain_inst.ins, ScopedClock({None: tick_clock.global_clock})
            )
            si = drain_inst.ins.sync_info
            waits = list(si.on_wait) if si and si.on_wait else []
            if len(waits) > 1:
                si.on_wait = waits[:1]
                for w in waits[1:]:
                    extra = nc.sync.drain()
                    esi = extra.ins.sync_info
                    if esi is None:
                        extra.ins.sync_info = mybir.SyncInfo(on_wait=[w], on_update=[])
                    else:
                        esi.on_wait.append(w)
            nc.all_engine_barrier()
            popped = nc._tile_sem_poison_stack.pop()
            assert popped is self._sem_poison
            nc.clear_and_free_semaphores(list(self.sems.allocated().values()))
            nc.all_engine_barrier()

    fp16 = mybir.dt.float16
    fp32 = mybir.dt.float32
    Relu = mybir.ActivationFunctionType.Relu
    Copy = mybir.ActivationFunctionType.Copy
    MUL = mybir.AluOpType.mult
    ADD = mybir.AluOpType.add

    sched = _schedule(t_row)

    nc = bass.Bass()
    d_acsT = nc.declare_dram_parameter("acsT", [AC + 1, G * GC], fp16, isOutput=False)
    d_obT = nc.declare_dram_parameter("obT", [OB + 1, BPC], fp16, isOutput=False)
    d_stac1 = nc.declare_dram_parameter("stac1", [AC + 1, W], fp16, isOutput=False)
    d_stac2 = nc.declare_dram_parameter("stac2", [W + 1, ACL], fp16, isOutput=False)
    d_stenc1 = nc.declare_dram_parameter("stenc1", [OB + 1, W], fp16, isOutput=False)
    d_stenc2 = nc.declare_dram_parameter("stenc2", [W + 1, OBL], fp16, isOutput=False)
    d_stdyn = nc.declare_dram_parameter("stdyn", [128, 576], fp16, isOutput=False)
    d_stl2 = nc.declare_dram_parameter("stl2", [W + 1, OBL], fp16, isOutput=False)
    d_stdelta = nc.declare_dram_parameter("stdelta", [128, 64], fp16, isOutput=False)
    d_stdec1 = nc.declare_dram_parameter("stdec1", [OBL + 1, W], fp16, isOutput=False)
    d_stdec2 = nc.declare_dram_parameter("stdec2", [W + 1, OB], fp16, isOutput=False)
    d_out = nc.declare_dram_parameter("out", [OB, G * GC], fp32, isOutput=True)

    with SplitDrainTileContext(nc) as tc:
        with tc.tile_pool(name="singles", bufs=1) as sp, \
             tc.tile_pool(name="psum", bufs=8, space="PSUM") as psum_pool, \
             tc.tile_pool(name="acst", bufs=2) as acst_pool, \
             tc.tile_pool(name="outs", bufs=4) as outs_pool:
            ACLT = [sp.tile([ACL, GC], fp16, name=f"aclt{g}") for g in range(G)]
            YS = [sp.tile([OBL + 1, GC], fp16, name=f"ys{g}") for g in range(G)]
            S1 = [sp.tile([128, GB], fp16, name=f"st1_{g}") for g in range(G)]
            S2 = [sp.tile([128, GB], fp16, name=f"st2_{g}") for g in range(G)]
            HT = [[sp.tile([W + 1, GB], fp16, name=f"h{g}_{p}") for p in range(2)]
                  for g in range(G)]
            YCUR = [sp.tile([OBL, GB], fp32, name=f"ycur{g}") for g in range(G)]
            HDEC = [[sp.tile([W + 1, 512], fp16, name=f"hdec{g}_{p}")
                     for p in range(2)] for g in range(G)]
            HAC = [sp.tile([W + 1, 512], fp16, name=f"hac{p}") for p in range(2)]
            HENC = sp.tile([W + 1, BPC], fp16, name="henc")
            OBT = sp.tile([OB + 1, BPC], fp16, name="obt")
            STDYN = sp.tile([128, 576], fp16, name="stdyn_t")
            STL2 = sp.tile([W + 1, OBL], fp16, name="stl2_t")
            STDELTA = sp.tile([128, 64], fp16, name="stdelta_t")
            STDEC1 = sp.tile([OBL + 1, W], fp16, name="stdec1_t")
            STDEC2 = sp.tile([W + 1, OB], fp16, name="stdec2_t")
            STAC1 = sp.tile([AC + 1, W], fp16, name="stac1_t")
            STAC2 = sp.tile([W + 1, ACL], fp16, name="stac2_t")
            STENC1 = sp.tile([OB + 1, W], fp16, name="stenc1_t")
            STENC2 = sp.tile([OB + 1, OBL], fp16, name="stenc2_t")

            for src, dst in [(d_stdyn, STDYN), (d_stl2, STL2),
                             (d_stdelta, STDELTA), (d_stdec1, STDEC1),
                             (d_stdec2, STDEC2), (d_stac1, STAC1), (d_stac2, STAC2),
                             (d_stenc1, STENC1), (d_stenc2, STENC2), (d_obT, OBT)]:
                nc.sync.dma_start(out=dst[:], in_=src[:])

            for g in range(G):
                nc.vector.memset(S1[g][:], 1.0)
                nc.vector.memset(S2[g][:], 0.0)
                nc.vector.memset(YS[g][OBL:OBL + 1, :], 1.0)
                for p in range(2):
                    nc.vector.memset(HT[g][p][W:W + 1, :], 1.0)
                    nc.vector.memset(HDEC[g][p][W:W + 1, :], 1.0)
            for p in range(2):
                nc.vector.memset(HAC[p][W:W + 1, :], 1.0)
            nc.vector.memset(HENC[W:W + 1, :], 1.0)

            # ---- action-latent phase ----
            blk = 0
            for c in range(4):
                for g in range(G):
                    at = acst_pool.tile([AC + 1, 4096], fp16, name="acst_t")
                    off = g * GC + c * 4096
                    nc.sync.dma_start(out=at[:], in_=d_acsT[0:AC + 1, off:off + 4096])
                    for b2 in range(8):
                        mv = at[0:AC + 1, b2 * 512:(b2 + 1) * 512]
                        p1 = psum_pool.tile([128, 512], fp32, name="ps")
                        nc.tensor.matmul(p1[0:W, 0:512], STAC1[:], mv,
                                         start=True, stop=True)
                        hb = HAC[blk % 2]
                        nc.scalar.activation(hb[0:W, :], p1[0:W, 0:512], Relu)
                        p2 = psum_pool.tile([128, 512], fp32, name="ps")
                        nc.tensor.matmul(p2[0:ACL, 0:512], STAC2[:], hb[:],
                                         start=True, stop=True)
                        dst = ACLT[g][0:ACL, c * 4096 + b2 * 512:
                                      c * 4096 + (b2 + 1) * 512]
                        if blk % 2 == 0:
                            nc.vector.tensor_copy(dst, p2[0:ACL, 0:512])
                        else:
                            nc.scalar.activation(dst, p2[0:ACL, 0:512], Copy,
                                                 bias=0.0)
                        blk += 1

            # ---- encoder phase ----
            pe1 = psum_pool.tile([128, 512], fp32, name="ps")
            nc.tensor.matmul(pe1[0:W, 0:BPC], STENC1[:], OBT[:],
                             start=True, stop=True)
            nc.scalar.activation(HENC[0:W, :], pe1[0:W, 0:BPC], Relu)
            pe2 = psum_pool.tile([128, 512], fp32, name="ps")
            nc.tensor.matmul(pe2[0:OBL, 0:BPC], STENC2[:], HENC[:],
                             start=True, stop=True)
            for g in range(G):
                seg = pe2[0:OBL, g * GB:(g + 1) * GB]
                nc.vector.tensor_copy(YCUR[g][:], seg)
                nc.scalar.activation(S1[g][0:OBL, :], seg, Copy, bias=0.0)
                nc.scalar.activation(YS[g][0:OBL, 0:GB], seg, Copy, bias=0.0)

            # ---- ODE loop with interleaved decode ----
            def emit_decode_block(c, b2, g):
                col = c * 4096 + b2 * 512
                pd1 = psum_pool.tile([128, 512], fp32, name="ps")
                nc.tensor.matmul(pd1[0:W, 0:512], STDEC1[:],
                                 YS[g][0:OBL + 1, col:col + 512],
                                 start=True, stop=True)
                hd = HDEC[g][b2 % 2]
                if g == 0:
                    nc.scalar.activation(hd[0:W, :], pd1[0:W, 0:512], Relu)
                else:
                    nc.vector.tensor_scalar_max(hd[0:W, :], pd1[0:W, 0:512], 0.0)
                pd2 = psum_pool.tile([128, 512], fp32, name="ps")
                nc.tensor.matmul(pd2[0:OB, 0:512], STDEC2[:], hd[:],
                                 start=True, stop=True)
                ot = outs_pool.tile([OB, 512], fp32, name="outs_t")
                if g == 0:
                    nc.vector.tensor_copy(ot[:], pd2[0:OB, 0:512])
                else:
                    nc.scalar.activation(ot[:], pd2[0:OB, 0:512], Copy, bias=0.0)
                nc.sync.dma_start(out=d_out[0:OB, g * GC + col:g * GC + col + 512],
                                  in_=ot[:])

            cur_idx = [None, None]
            pending = []
            for (i, j, h_f, idxs) in sched:
                for s in range(6):
                    for g in range(G):
                        if idxs[s] != cur_idx[g]:
                            ix = idxs[s]
                            nc.gpsimd.tensor_copy(
                                S1[g][OBL:OBL + ACL, :],
                                ACLT[g][0:ACL, ix * GB:(ix + 1) * GB])
                            cur_idx[g] = ix
                        p1 = psum_pool.tile([128, 512], fp32, name="ps")
                        pa = p1[0:W, 0:GB]
                        if B_ROWS[s] == 0:
                            nc.tensor.matmul(
                                pa, STDYN[0:SROWS[s], s * 64:(s + 1) * 64],
                                S1[g][0:SROWS[s], :], start=True, stop=True)
                        else:
                            nc.tensor.matmul(
                                pa, STDYN[0:128, s * 64:(s + 1) * 64],
                                S1[g][0:128, :], start=True, stop=False)
                            nc.tensor.matmul(
                                pa, STDYN[0:B_ROWS[s], B_COL[s]:B_COL[s] + 64],
                                S2[g][0:B_ROWS[s], :], start=False, stop=True)
                        hb = HT[g][s % 2]
                        if g == 0:
                            nc.scalar.activation(hb[0:W, :], pa, Relu)
                        else:
                            nc.vector.tensor_scalar_max(hb[0:W, :], pa, 0.0)
                        p2 = psum_pool.tile([128, 512], fp32, name="ps")
                        pk = p2[0:OBL, 0:GB]
                        nc.tensor.matmul(pk, STL2[:], hb[:], start=True, stop=True)
                        if s < 2:
                            kdst = S1[g][64 + 32 * s:64 + 32 * (s + 1), :]
                        else:
                            kdst = S2[g][32 * (s - 2):32 * (s - 1), :]
                        if g == 0:
                            nc.vector.tensor_scalar_mul(kdst, pk, h_f)
                        else:
                            nc.scalar.activation(kdst, pk, Copy, scale=h_f)
                for g in range(G):
                    pD = psum_pool.tile([128, 512], fp32, name="ps")
                    pd = pD[0:OBL, 0:GB]
                    nc.tensor.matmul(pd, STDELTA[64:96, 32:64], S1[g][64:96, :],
                                     start=True, stop=False)
                    nc.tensor.matmul(pd, STDELTA[0:128, 0:32], S2[g][0:128, :],
                                     start=False, stop=True)
                    nc.vector.scalar_tensor_tensor(
                        YCUR[g][:], pd, 1.0, YCUR[g][:], MUL, ADD)
                    nc.gpsimd.tensor_copy(S1[g][0:OBL, :], YCUR[g][:])
                    if j == 1:
                        nc.gpsimd.tensor_copy(
                            YS[g][0:OBL, (i + 1) * GB:(i + 2) * GB], YCUR[g][:])
                if j == 1 and i in (30, 62, 94, 126):
                    c = (i - 30) // 32
                    pending += [(c, b2, g) for b2 in range(8) for g in range(G)]
                if pending and i < 126:
                    emit_decode_block(*pending.pop(0))
            while pending:
                emit_decode_block(*pending.pop(0))

    return nc


def _get_program(t_row: np.ndarray):
    key = t_row.astype(np.float32).tobytes()
    if key not in _prog_cache:
        _prog_cache[key] = _build_program(t_row)
    return _prog_cache[key]


def _stationaries(inputs):
    f64 = np.float64

    def cat_wb(Wm, b):
        return np.concatenate([np.asarray(Wm, f64).T,
                               np.asarray(b, f64)[None]], 0)

    st = {
        "stac1": cat_wb(inputs["acW0"], inputs["acb0"]).astype(F16),
        "stac2": cat_wb(inputs["acW1"], inputs["acb1"]).astype(F16),
        "stenc1": cat_wb(inputs["encW0"], inputs["encb0"]).astype(F16),
        "stenc2": cat_wb(inputs["encW1"], inputs["encb1"]).astype(F16),
        "stl2": cat_wb(inputs["dynW1"], inputs["dynb1"]).astype(F16),
        "stdec1": cat_wb(inputs["decW0"], inputs["decb0"]).astype(F16),
        "stdec2": cat_wb(inputs["decW1"], inputs["decb1"]).astype(F16),
    }
    dynW0 = np.asarray(inputs["dynW0"], f64)
    dynb0 = np.asarray(inputs["dynb0"], f64)
    W0yT = dynW0[:, :OBL].T                      # (32, 64)
    W0aT = dynW0[:, OBL:].T                      # (16, 64)
    base = np.concatenate([W0yT, W0aT, dynb0[None]], 0)   # (49, 64)
    stdyn = np.zeros((128, 576), f64)
    for s in range(6):
        stdyn[0:49, s * 64:(s + 1) * 64] = base
        for q, a in enumerate(DOPRI_A[s][:2]):
            stdyn[64 + 32 * q:96 + 32 * q, s * 64:(s + 1) * 64] = a * W0yT
        if B_ROWS[s]:
            brows = np.concatenate([a * W0yT for a in DOPRI_A[s][2:]], 0)
            stdyn[0:brows.shape[0], B_COL[s]:B_COL[s] + 64] = brows
    st["stdyn"] = stdyn.astype(F16)
    eye = np.eye(OBL, dtype=f64)
    stdelta = np.zeros((128, 64), f64)
    for q, s in enumerate((2, 3, 4, 5)):
        stdelta[32 * q:32 * (q + 1), 0:32] = DOPRI_BW[s] * eye
    stdelta[64:96, 32:64] = DOPRI_BW[0] * eye
    st["stdelta"] = stdelta.astype(F16)
    return st


def _make_in_maps(inputs):
    f32 = np.float32
    acs = np.asarray(inputs["acs"], f32)
    ob = np.asarray(inputs["ob"], f32)
    st = _stationaries(inputs)

    in_maps = []
    for c in range(N_CORES):
        sl = slice(c * BPC, (c + 1) * BPC)
        a = acs[sl]
        acsT = np.empty((AC + 1, G * GC), F16)
        for g in range(G):
            blk = a[g * GB:(g + 1) * GB].transpose(2, 1, 0).reshape(AC, GC)
            acsT[0:AC, g * GC:(g + 1) * GC] = blk.astype(F16)
        acsT[AC, :] = 1.0
        obT = np.empty((OB + 1, BPC), F16)
        obT[0:OB] = ob[sl].T.astype(F16)
        obT[OB] = 1.0
        in_maps.append({"acsT": acsT, "obT": obT, **st})
    return in_maps


def _unshard_core(o: np.ndarray) -> np.ndarray:
    return (np.asarray(o).reshape(OB, G, T, GB)
            .transpose(1, 3, 2, 0).reshape(BPC, T, OB))


def kernel(**inputs) -> np.ndarray:
    from concourse.bass_utils import run_bass_kernel_spmd

    f32 = np.float32
    times = np.asarray(inputs["times"], f32)
    nc = _get_program(times[0])
    in_maps = _make_in_maps(inputs)

    res = run_bass_kernel_spmd(nc, in_maps, core_ids=list(range(N_CORES)))

    out = np.empty((B, T, OB), f32)
    for c in range(N_CORES):
        out[c * BPC:(c + 1) * BPC] = _unshard_core(res.results[c]["out"])
    return out
